# revision 14
# baseline (speedup 1.0000x reference)
"""Gemma3 single-token decode on 8 trn2 NeuronCores (tensor-parallel SPMD).

Sharding: attention by head (pairs of cores compute the same head redundantly,
Wo pre-scaled by 0.5 so the 8-way AllReduce sums correctly); FFN 8-way over the
FF dim; lm_head 8-way over vocab with host-side final argmax; KV cache sliced
to the live prefix and replicated; norms computed on every core.

All matvecs use moving-weight matmuls (activation stationary), activations in
fp32, weights optionally bf16 (KBF16=1).

Execution layer: one persistent jit(shard_map(bass_exec)) callable is built per
process; the prepped weight shards are staged onto the 8 cores once (content-
fingerprint cached) and stay resident, so steady-state kernel() calls ship only
the per-token tensors (embedding row, rope row, masks) and fetch the sharded
logits back.
"""
import sys, os
sys.path.insert(0, '/opt/trn_rl_repo')
import numpy as np
import ml_dtypes

import concourse.bass as bass
import concourse.bacc as bacc
import concourse.mybir as mybir
import concourse.tile as tile

L, HID, NCH, D, H, FF, VOCAB = 12, 1152, 9, 256, 4, 6912, 64000
FSH = FF // 8            # 864 ffn rows per core
VS = VOCAB // 8          # 8000 vocab rows per core
SEFF, T = 1024, 8        # live kv prefix (pos=1000 -> 1024), 8 s-tiles
SCALE, EPS = 256.0 ** -0.5, 1e-6
NC_ = 8
F32 = mybir.dt.float32
AF = mybir.ActivationFunctionType
X_AX = mybir.AxisListType.X

BF16 = os.environ.get("KBF16", "1") == "1"
_SIMGELU = os.environ.get("KSIMGELU", "0") == "1"  # CoreSim lacks Gelu_apprx_tanh
WNP = ml_dtypes.bfloat16 if BF16 else np.float32


def _build(wdt):
    nc = bacc.Bacc("TRN2", target_bir_lowering=False, debug=False, num_devices=NC_)
    _eps_t = nc.alloc_sbuf_tensor("const-eps", [128, 1], F32)
    nc.gpsimd.memset(_eps_t.ap(), EPS)
    nc.const_aps.aps[(F32, EPS)] = _eps_t.ap()
    nc.all_engine_barrier()

    def dI(n, sh, dt=F32):
        return nc.dram_tensor(n, sh, dt, kind="ExternalInput").ap()

    h0row = dI("h0row", [1, HID])
    cs = dI("cs", [1, 1024])
    mcol = dI("mcol", [128, 40])
    um_w = dI("um_w", [128, 8], wdt)
    wqkv = dI("wqkv", [L, 3, 128, 2304], wdt)
    wo = dI("wo", [L, 128, 2, HID], wdt)
    ktd = dI("kt", [L, 128, 2, SEFF], wdt)
    vcd = dI("vc", [L, 128, T, D], wdt)
    wgd = dI("wg", [L, 3, 128, 2592], wdt)
    wud = dI("wu", [L, 3, 128, 2592], wdt)
    wdd = dI("wd", [L, 128, 7, HID], wdt)
    lmd = dI("lm", [NCH, 128, VS], wdt)
    logits = nc.dram_tensor("logits", [1, VS], F32, kind="ExternalOutput").ap()

    with tile.TileContext(nc) as tc, \
         tc.tile_pool(name="const", bufs=1) as Pc, \
         tc.tile_pool(name="wqkv", bufs=3) as Pwq, \
         tc.tile_pool(name="wo", bufs=1) as Pwo, \
         tc.tile_pool(name="kt", bufs=1) as Pkt, \
         tc.tile_pool(name="vc", bufs=1) as Pvc, \
         tc.tile_pool(name="wg", bufs=3) as Pwg, \
         tc.tile_pool(name="wu", bufs=3) as Pwu, \
         tc.tile_pool(name="wd", bufs=3) as Pwd, \
         tc.tile_pool(name="lm", bufs=4) as Plm, \
         tc.tile_pool(name="act", bufs=2) as Pa, \
         tc.tile_pool(name="row", bufs=3) as Pr, \
         tc.tile_pool(name="ps", bufs=2, space="PSUM") as Pp, \
         tc.tile_pool(name="dram", bufs=2, space="DRAM") as Pd:

        MM = nc.tensor.matmul
        one_f = Pc.tile([1, 1], F32, tag="onef")
        nc.vector.memset(one_f[:], 1.0)
        one_w = Pc.tile([1, 1], wdt, tag="onew")
        nc.vector.memset(one_w[:], 1.0)
        ones_cf = Pc.tile([128, 1], F32, tag="ocf")
        nc.vector.memset(ones_cf[:], 1.0)
        ones_row = Pc.tile([1, 128], F32, tag="orow")
        nc.vector.memset(ones_row[:], 1.0)
        cs_t = Pc.tile([1, 1024], F32, tag="cs")
        nc.sync.dma_start(out=cs_t[:], in_=cs[:])
        mc = Pc.tile([128, 40], F32, tag="mc")
        nc.sync.dma_start(out=mc[:], in_=mcol[:])
        umw_t = Pc.tile([128, 8], wdt, tag="umw")
        nc.sync.dma_start(out=umw_t[:], in_=um_w[:])
        ADDM, VM, VMU, UM1, UMF = (mc[:, 8 * i:8 * i + 8] for i in range(5))

        def cast_col(src_t, tag):
            if wdt == F32:
                return src_t
            w = Pa.tile([128, NCH], wdt, tag=tag)
            nc.vector.tensor_copy(w[:], src_t[:])
            return w

        def columnize(row_ap, n, one_t, PS, base):
            ps = PS[:, base:base + n]
            for j in range(n):
                MM(ps[:, j:j + 1], row_ap[0:1, j * 128:(j + 1) * 128], one_t[:],
                   start=True, stop=True)
            return ps

        def rms_col(h_t, tag, PS, base):
            sq = Pa.tile([128, NCH], F32, tag="sq")
            nc.vector.tensor_mul(sq[:], h_t[:], h_t[:])
            MM(PS[0:1, base:base + NCH], ones_cf[:], sq[:], start=True, stop=True)
            st = Pa.tile([1, 4], F32, tag="rmsst")
            nc.vector.reduce_sum(st[0:1, 0:1], PS[0:1, base:base + NCH], axis=X_AX)
            nc.scalar.activation(st[0:1, 1:2], st[0:1, 0:1], AF.Sqrt,
                                 bias=EPS, scale=1.0 / HID)
            nc.vector.reciprocal(st[0:1, 2:3], st[0:1, 1:2])
            rb = PS[:, 96 + base:97 + base]
            MM(rb, ones_row[:], st[0:1, 2:3], start=True, stop=True)
            x = Pa.tile([128, NCH], F32, tag=tag)
            nc.vector.tensor_scalar_mul(x[:], h_t[:], rb)
            return x

        def resid_add(h_t, row_t, PS):
            st = Pa.tile([1, 4], F32, tag="rmsst")
            scr = Pr.tile([1, HID], F32, tag="r1152")
            nc.scalar.activation(scr[:], row_t[:], AF.Square,
                                 accum_out=st[0:1, 0:1])
            nc.scalar.activation(st[0:1, 1:2], st[0:1, 0:1], AF.Sqrt,
                                 bias=EPS, scale=1.0 / HID)
            nc.vector.reciprocal(st[0:1, 2:3], st[0:1, 1:2])
            rb = PS[:, 74:75]
            MM(rb, ones_row[:], st[0:1, 2:3], start=True, stop=True)
            pc = columnize(row_t, NCH, one_f, PS, 64)
            tmp = Pa.tile([128, NCH], F32, tag="tmph")
            nc.vector.tensor_scalar_mul(tmp[:], pc[:], rb)
            hn = Pa.tile([128, NCH], F32, tag="h")
            nc.vector.tensor_add(hn[:], h_t[:], tmp[:])
            return hn

        NOAR = os.environ.get("KNOAR", "0") == "1"       # timing probe only
        USE_AG = os.environ.get("KAG", "0") == "1"       # allgather+local reduce

        def all_reduce(row_t):
            bin_ = Pd.tile([1, HID], F32, tag="arin")
            nc.scalar.dma_start(out=bin_[:], in_=row_t[:])
            if NOAR:
                ar = Pr.tile([1, HID], F32, tag="r1152")
                nc.scalar.dma_start(out=ar[:], in_=bin_[:])
                return ar
            if USE_AG:
                bout = Pd.tile([8, HID], F32, tag="arout8")
                nc.gpsimd.collective_compute(
                    "AllGather", mybir.AluOpType.bypass,
                    replica_groups=[list(range(NC_))],
                    ins=[bin_.opt()], outs=[bout.opt()])
                gath = Pa.tile([8, HID], F32, tag="gath")
                nc.sync.dma_start(out=gath[:], in_=bout[:])
                pr = Pp.tile([1, HID], F32, tag="pbig", name=None)
                MM(pr[0:1, 0:512], ones_cf[0:8, :], gath[:, 0:512],
                   start=True, stop=True)
                MM(pr[0:1, 512:1024], ones_cf[0:8, :], gath[:, 512:1024],
                   start=True, stop=True)
                MM(pr[0:1, 1024:1152], ones_cf[0:8, :], gath[:, 1024:1152],
                   start=True, stop=True)
                ar = Pr.tile([1, HID], F32, tag="r1152")
                nc.scalar.activation(ar[:], pr[0:1, :], AF.Copy)
                return ar
            bout = Pd.tile([1, HID], F32, tag="arout")
            nc.gpsimd.collective_compute(
                "AllReduce", mybir.AluOpType.add,
                replica_groups=[list(range(NC_))],
                ins=[bin_.opt()], outs=[bout.opt()])
            ar = Pr.tile([1, HID], F32, tag="r1152")
            nc.scalar.dma_start(out=ar[:], in_=bout[:])
            return ar

        # h0: [1,1152] row -> column layout
        h0r = Pr.tile([1, HID], F32, tag="r1152")
        nc.sync.dma_start(out=h0r[:], in_=h0row[:])
        PS = Pp.tile([128, 512], F32, tag="psmall")
        pc0 = columnize(h0r, NCH, one_f, PS, 64)
        h = Pa.tile([128, NCH], F32, tag="h")
        nc.scalar.activation(h[:], pc0[:], AF.Copy)

        for l in range(L):
            # ---- attention ----
            PS = Pp.tile([128, 512], F32, tag="psmall")
            x = rms_col(h, "x", PS, 0)
            xw = cast_col(x, "xw")
            pqkv = Pp.tile([1, 1152], F32, tag="pbig")
            for g in range(3):
                wt = Pwq.tile([128, 2304], wdt, tag="wqkv")
                nc.sync.dma_start(out=wt[:], in_=wqkv[l, g])
                for ci in range(3):
                    c = g * 3 + ci
                    for n0, ln in ((0, 512), (512, 256)):
                        MM(pqkv[0:1, n0:n0 + ln], xw[:, c:c + 1],
                           wt[:, ci * 768 + n0: ci * 768 + n0 + ln],
                           start=(c == 0), stop=(c == 8))
            # q/k rms over D (rows on partition 0)
            st = Pa.tile([1, 6], F32, tag="qkst")
            scr = Pr.tile([1, 256], F32, tag="r256")
            nc.scalar.activation(scr[:], pqkv[0:1, 0:256], AF.Square,
                                 accum_out=st[0:1, 0:1])
            scr2 = Pr.tile([1, 256], F32, tag="r256")
            nc.scalar.activation(scr2[:], pqkv[0:1, 256:512], AF.Square,
                                 accum_out=st[0:1, 1:2])
            nc.scalar.activation(st[0:1, 2:3], st[0:1, 0:1], AF.Sqrt,
                                 bias=EPS, scale=1.0 / D)
            nc.scalar.activation(st[0:1, 3:4], st[0:1, 1:2], AF.Sqrt,
                                 bias=EPS, scale=1.0 / D)
            nc.vector.reciprocal(st[0:1, 4:5], st[0:1, 2:3])
            nc.vector.reciprocal(st[0:1, 5:6], st[0:1, 3:4])
            cof = 512 if ((l + 1) % 6 == 0) else 0
            cosr = cs_t[0:1, cof:cof + 256]
            sinr = cs_t[0:1, cof + 256:cof + 512]

            def rope(off, rinv, tag):
                t1 = Pr.tile([1, 256], F32, tag="ropet")
                nc.vector.tensor_mul(t1[:], pqkv[0:1, off:off + 256], cosr)
                sw = Pr.tile([1, 256], F32, tag="ropes")
                nc.vector.tensor_copy(sw[0:1, 0:128], pqkv[0:1, off + 128:off + 256])
                nc.vector.tensor_copy(sw[0:1, 128:256], pqkv[0:1, off:off + 128])
                nc.vector.tensor_mul(sw[:], sw[:], sinr)
                nc.vector.tensor_add(t1[:], t1[:], sw[:])
                out = Pr.tile([1, 256], F32, tag=tag)
                nc.vector.tensor_scalar_mul(out[:], t1[:], rinv)
                return out

            qr = rope(0, st[0:1, 4:5], "qr")
            kr = rope(256, st[0:1, 5:6], "kr")
            # columnize q,k -> [128,2] each (wdt)
            pqc = PS[:, 88:92]
            for j in range(2):
                MM(pqc[:, j:j + 1], qr[0:1, j * 128:(j + 1) * 128], one_f[:],
                   start=True, stop=True)
                MM(pqc[:, 2 + j:3 + j], kr[0:1, j * 128:(j + 1) * 128], one_f[:],
                   start=True, stop=True)
            qkc = Pa.tile([128, 4], wdt, tag="qkc")
            nc.scalar.activation(qkc[:], pqc[:], AF.Copy)

            # scores^T [128, T] (s = t*128 + r)
            kt_t = Pkt.tile([128, 2, SEFF], wdt, tag="kt")
            nc.sync.dma_start(out=kt_t[:], in_=ktd[l])
            psc = PS[:, 80:88]
            for t_ in range(T):
                for c in range(2):
                    MM(psc[:, t_:t_ + 1],
                       kt_t[:, c, t_ * 128: t_ * 128 + 128],
                       qkc[:, c:c + 1], start=(c == 0), stop=(c == 1))
            # qk_new = q . k_new
            pqk = PS[0:1, 18:48]
            for c in range(2):
                MM(pqk[0:1, 10:11], qkc[:, c:c + 1], qkc[:, 2 + c:3 + c],
                   start=(c == 0), stop=(c == 1))
            qks = Pa.tile([1, 1], F32, tag="qks")
            nc.scalar.activation(qks[:], pqk[0:1, 10:11], AF.Copy)
            bq = PS[:, 78:79]
            MM(bq, ones_row[:], qks[:], start=True, stop=True)
            # fix scores at s=p, scale, mask, clamp, exp
            sc1 = Pa.tile([128, T], F32, tag="sc1")
            nc.vector.tensor_mul(sc1[:], psc[:], UM1)
            sc2 = Pa.tile([128, T], F32, tag="sc2")
            nc.vector.tensor_scalar_mul(sc2[:], UMF, bq)
            nc.vector.tensor_add(sc1[:], sc1[:], sc2[:])
            nc.vector.tensor_scalar_mul(sc1[:], sc1[:], float(SCALE))
            nc.vector.tensor_add(sc1[:], sc1[:], ADDM)
            nc.vector.tensor_scalar_max(sc1[:], sc1[:], -30.0)
            probs = Pa.tile([128, T], F32, tag="probs")
            nc.scalar.activation(probs[:], sc1[:], AF.Exp)
            # denominator and p_at_update (f32)
            pmf = Pa.tile([128, T], F32, tag="pmf")
            nc.vector.tensor_mul(pmf[:], probs[:], VM)
            puf = Pa.tile([128, T], F32, tag="puf")
            nc.vector.tensor_mul(puf[:], probs[:], UMF)
            MM(pqk[0:1, 0:8], ones_cf[:], pmf[:], start=True, stop=True)
            psums = Pa.tile([1, 8], F32, tag="psums")
            nc.scalar.activation(psums[:], pqk[0:1, 0:8], AF.Copy)
            MM(pqk[0:1, 8:10], ones_cf[:], puf[:, 0:2], start=True, stop=False)
            MM(pqk[0:1, 8:10], ones_cf[:], puf[:, 2:4], start=False, stop=False)
            MM(pqk[0:1, 8:10], ones_cf[:], puf[:, 4:6], start=False, stop=False)
            MM(pqk[0:1, 8:10], ones_cf[:], puf[:, 6:8], start=False, stop=True)
            dn = Pa.tile([1, 4], F32, tag="dn")
            nc.vector.reduce_sum(dn[0:1, 0:1], psums[0:1, 0:8], axis=X_AX)
            nc.vector.reciprocal(dn[0:1, 1:2], dn[0:1, 0:1])
            nc.vector.reduce_sum(dn[0:1, 2:3], pqk[0:1, 8:10], axis=X_AX)
            # o = (probs_masked @ V + pu*v_new) / den
            pmv = Pa.tile([128, T], wdt, tag="pmv")
            nc.vector.tensor_mul(pmv[:], probs[:], VMU)
            vc_t = Pvc.tile([128, T, D], wdt, tag="vc")
            nc.sync.dma_start(out=vc_t[:], in_=vcd[l])
            po = PS[0:1, 128:384]
            for t_ in range(T):
                MM(po[0:1, 0:256], pmv[:, t_:t_ + 1], vc_t[:, t_, :],
                   start=(t_ == 0), stop=(t_ == T - 1))
            vv = Pr.tile([1, 256], F32, tag="vv")
            nc.vector.tensor_scalar_mul(vv[:], pqkv[0:1, 512:768], dn[0:1, 2:3])
            ofin = Pr.tile([1, 256], F32, tag="ofin")
            nc.vector.tensor_add(ofin[:], po[0:1, 0:256], vv[:])
            nc.vector.tensor_scalar_mul(ofin[:], ofin[:], dn[0:1, 1:2])
            # Wo partial (pre-scaled 0.5 on host)
            poc = PS[:, 92:96]
            for j in range(2):
                MM(poc[:, j:j + 1], ofin[0:1, j * 128:(j + 1) * 128], one_f[:],
                   start=True, stop=True)
            ocol = Pa.tile([128, 2], wdt, tag="ocol")
            nc.scalar.activation(ocol[:], poc[:, 92 - 92:94 - 92], AF.Copy)
            wo_t = Pwo.tile([128, 2, HID], wdt, tag="wo")
            nc.sync.dma_start(out=wo_t[:], in_=wo[l])
            prow = Pp.tile([1, HID], F32, tag="pbig")
            for c in range(2):
                for n0, ln in ((0, 512), (512, 512), (1024, 128)):
                    MM(prow[0:1, n0:n0 + ln], ocol[:, c:c + 1],
                       wo_t[:, c, n0: n0 + ln],
                       start=(c == 0), stop=(c == 1))
            arow = Pr.tile([1, HID], F32, tag="r1152")
            nc.scalar.activation(arow[:], prow[0:1, :], AF.Copy)
            ar1 = all_reduce(arow)
            h = resid_add(h, ar1, PS)

            # ---- ffn ----
            x2 = rms_col(h, "x2", PS, 9)
            x2w = cast_col(x2, "x2w")
            pg = Pp.tile([1, FSH], F32, tag="pbig", padded_shape=[1, HID])
            pu_ = Pp.tile([1, FSH], F32, tag="pbig", padded_shape=[1, HID])
            for g in range(3):
                wg_t = Pwg.tile([128, 2592], wdt, tag="wg")
                nc.sync.dma_start(out=wg_t[:], in_=wgd[l, g])
                wu_t = Pwu.tile([128, 2592], wdt, tag="wu")
                nc.sync.dma_start(out=wu_t[:], in_=wud[l, g])
                for ci in range(3):
                    c = g * 3 + ci
                    for n0, ln in ((0, 512), (512, 352)):
                        MM(pg[0:1, n0:n0 + ln], x2w[:, c:c + 1],
                           wg_t[:, ci * FSH + n0: ci * FSH + n0 + ln],
                           start=(c == 0), stop=(c == 8))
                        MM(pu_[0:1, n0:n0 + ln], x2w[:, c:c + 1],
                           wu_t[:, ci * FSH + n0: ci * FSH + n0 + ln],
                           start=(c == 0), stop=(c == 8))
            gact = Pr.tile([1, FSH], F32, tag="gact")
            nc.scalar.activation(gact[:], pg[0:1, :],
                     AF.Tanh if _SIMGELU else AF.Gelu_apprx_tanh)
            prod = Pr.tile([1, 896], wdt, tag="prod")
            nc.vector.memset(prod[0:1, FSH:896], 0.0)
            nc.vector.tensor_mul(prod[0:1, 0:FSH], gact[:], pu_[0:1, :])
            pcd = columnize(prod, 7, one_w, PS, 64)
            pdc = Pa.tile([128, 7], wdt, tag="pdc")
            nc.scalar.activation(pdc[:], pcd[:], AF.Copy)
            pf = Pp.tile([1, HID], F32, tag="pbig")
            for s_ in range(4):
                if s_ < 3:
                    wd_t = Pwd.tile([128, 2, HID], wdt, tag="wd")
                    nc.sync.dma_start(out=wd_t[:],
                                      in_=wdd[l, :, 2 * s_:2 * s_ + 2])
                else:
                    wd_t = Pwd.tile([128, 1, HID], wdt, tag="wd")
                    nc.sync.dma_start(out=wd_t[:], in_=wdd[l, :, 6:7])
                for fi in range(2 if s_ < 3 else 1):
                    fc = 2 * s_ + fi
                    for n0, ln in ((0, 512), (512, 512), (1024, 128)):
                        MM(pf[0:1, n0:n0 + ln], pdc[:, fc:fc + 1],
                           wd_t[:, fi, n0: n0 + ln],
                           start=(fc == 0), stop=(fc == 6))
            frow = Pr.tile([1, HID], F32, tag="r1152")
            nc.scalar.activation(frow[:], pf[0:1, :], AF.Copy)
            ar2 = all_reduce(frow)
            h = resid_add(h, ar2, PS)

        # ---- final norm + lm_head (vocab shard) ----
        PSf = Pp.tile([128, 512], F32, tag="psmall")
        xf = rms_col(h, "xf", PSf, 0)
        xfw = cast_col(xf, "xfw")
        for qt in range(4):
            pva = Pp.tile([1, HID], F32, tag="pbig", name=f"pva{qt}")
            pvb = Pp.tile([1, HID], F32, tag="pbig", name=f"pvb{qt}")
            regs = [pva[0:1, 0:500], pva[0:1, 512:1012],
                    pvb[0:1, 0:500], pvb[0:1, 512:1012]]
            for c in range(NCH):
                lm_t = Plm.tile([128, 2000], wdt, tag="lm")
                nc.sync.dma_start(out=lm_t[:],
                                  in_=lmd[c, :, qt * 2000:(qt + 1) * 2000])
                for vi in range(4):
                    MM(regs[vi], xfw[:, c:c + 1],
                       lm_t[:, vi * 500:(vi + 1) * 500],
                       start=(c == 0), stop=(c == NCH - 1))
            for vi in range(4):
                vg = qt * 4 + vi
                lrow = Pr.tile([1, 500], F32, tag="lrow")
                nc.scalar.activation(lrow[:], regs[vi], AF.Copy)
                nc.scalar.dma_start(out=logits[0:1, vg * 500:(vg + 1) * 500],
                                    in_=lrow[:])

    nc.compile()
    return nc


# ---------------------------------------------------------------------------
# host prep
# ---------------------------------------------------------------------------

def _to_w(x):
    """f32 ndarray -> weight dtype (ml_dtypes astype is SIMD-fast)."""
    if not BF16:
        return np.ascontiguousarray(x, np.float32)
    return np.ascontiguousarray(x, np.float32).astype(WNP)


def _grp3(wT, width):
    """[L,1152,width] -> [L,3,128,3*width] (any dtype)."""
    return np.ascontiguousarray(
        wT.reshape(L, 3, 3, 128, width).transpose(0, 1, 3, 2, 4)
    ).reshape(L, 3, 128, 3 * width)


def _prep_weights(inp):
    """Full weight set -> dict of GLOBAL arrays [8*d0, ...] ready to shard."""
    f32 = np.float32
    Wq = _to_w(inp['Wq'])            # [L,1024,1152]
    Wk = _to_w(inp['Wk'])            # [L,256,1152]
    Wv = _to_w(inp['Wv'])
    Wo = _to_w(inp['Wo'].astype(f32) * f32(0.5))   # [L,1152,1024]
    Wg = _to_w(inp['Wg'])            # [L,6912,1152]
    Wu = _to_w(inp['Wu'])
    Wd = _to_w(inp['Wd'])            # [L,1152,6912]
    lm = _to_w(inp['lm_head'])       # [VOCAB,1152]
    kvc = inp['kv_cache']

    g = {
        "wqkv": np.empty((NC_ * L, 3, 128, 2304), WNP),
        "wo": np.empty((NC_ * L, 128, 2, HID), WNP),
        "kt": np.empty((NC_ * L, 128, 2, SEFF), WNP),
        "vc": np.empty((NC_ * L, 128, T, D), WNP),
        "wg": np.empty((NC_ * L, 3, 128, 2592), WNP),
        "wu": np.empty((NC_ * L, 3, 128, 2592), WNP),
        "wd": np.empty((NC_ * L, 128, 7, HID), WNP),
        "lm": np.empty((NC_ * NCH, 128, VS), WNP),
    }

    # shared KV slices (replicated on every core)
    Kc = kvc[0:L, 0, 0:SEFF, :]                        # [L,S,D] f32
    kt1 = _to_w(np.ascontiguousarray(
        Kc.transpose(0, 2, 1).reshape(L, 2, 128, SEFF).transpose(0, 2, 1, 3)))
    vc1 = _to_w(np.ascontiguousarray(
        kvc[L:2 * L, 0, 0:SEFF, :].reshape(L, T, 128, D).transpose(0, 2, 1, 3)))
    for c in range(NC_):
        g["kt"][c * L:(c + 1) * L] = kt1
        g["vc"][c * L:(c + 1) * L] = vc1

    # attention shards: 4 distinct (head hd = c % 4), reused by core pairs
    for hd in range(4):
        wcat = np.concatenate([Wq[:, hd * D:(hd + 1) * D, :], Wk, Wv], axis=1)
        wqkv1 = _grp3(np.ascontiguousarray(wcat.transpose(0, 2, 1)), 768)
        wo1 = np.ascontiguousarray(
            Wo[:, :, hd * D:(hd + 1) * D].transpose(0, 2, 1)
            .reshape(L, 2, 128, HID).transpose(0, 2, 1, 3))
        for c in (hd, hd + 4):
            g["wqkv"][c * L:(c + 1) * L] = wqkv1
            g["wo"][c * L:(c + 1) * L] = wo1

    # ffn + lm_head shards: distinct per core
    for c in range(NC_):
        sl = slice(c * FSH, (c + 1) * FSH)
        g["wg"][c * L:(c + 1) * L] = _grp3(
            np.ascontiguousarray(Wg[:, sl, :].transpose(0, 2, 1)), FSH)
        g["wu"][c * L:(c + 1) * L] = _grp3(
            np.ascontiguousarray(Wu[:, sl, :].transpose(0, 2, 1)), FSH)
        wdT = np.zeros((L, 896, HID), WNP)
        wdT[:, :FSH, :] = Wd[:, :, sl].transpose(0, 2, 1)
        g["wd"][c * L:(c + 1) * L] = np.ascontiguousarray(
            wdT.reshape(L, 7, 128, HID).transpose(0, 2, 1, 3))
        g["lm"][c * NCH:(c + 1) * NCH] = np.ascontiguousarray(
            lm[c * VS:(c + 1) * VS, :].T).reshape(NCH, 128, VS)
    return g


def _prep_small(inp):
    """Per-token tensors -> dict of GLOBAL arrays (replicated across cores)."""
    f32 = np.float32
    p = int(np.asarray(inp['position_ids'])[0])
    tok = int(np.asarray(inp['input_ids'])[0])
    assert p + 1 <= SEFF, f"position {p} exceeds compiled kv window {SEFF}"

    h0 = (np.asarray(inp['embed'][tok]).astype(f32)
          * f32(HID ** 0.5)).reshape(1, HID)

    def sinsig(s):
        s = np.asarray(s)
        return np.concatenate([-s[0:128], s[128:256]])

    cs = np.concatenate([
        np.asarray(inp['cos_sliding'][p]), sinsig(inp['sin_sliding'][p]),
        np.asarray(inp['cos_full'][p]), sinsig(inp['sin_full'][p])
    ]).astype(f32).reshape(1, 1024)

    cm = np.asarray(inp['causal_mask'][:SEFF]).astype(f32)
    um = np.asarray(inp['update_mask'][:SEFF, 0]).astype(f32)
    col = lambda a: np.ascontiguousarray(a.reshape(T, 128).T)
    addm, umc = col(cm), col(um)
    vm = (addm > -1.0).astype(f32)
    mcol = np.concatenate([addm, vm, vm * (1 - umc), 1 - umc, umc],
                          axis=1).astype(f32)
    um_w = umc.astype(WNP)

    def rep(a):
        return np.ascontiguousarray(
            np.broadcast_to(a, (NC_, *a.shape))).reshape(NC_ * a.shape[0],
                                                         *a.shape[1:])

    return {"h0row": rep(h0), "cs": rep(cs), "mcol": rep(mcol),
            "um_w": rep(um_w)}


_HEAVY_IN = ("Wq", "Wk", "Wv", "Wo", "Wg", "Wu", "Wd", "lm_head", "kv_cache")


def _fingerprint(inp):
    """Content fingerprint of the weight tensors (strided sample + edges)."""
    parts = []
    for k in _HEAVY_IN:
        a = np.asarray(inp[k])
        v = a.reshape(-1).view(np.uint64) if a.nbytes % 8 == 0 else \
            a.reshape(-1).view(np.uint8)
        s = int(v[::61].sum(dtype=np.uint64))
        e = int(v[:512].sum(dtype=np.uint64)) ^ int(v[-512:].sum(dtype=np.uint64))
        parts.append((k, a.shape, a.dtype.str, s, e))
    return tuple(parts)


# ---------------------------------------------------------------------------
# persistent executor: one jit(shard_map(bass_exec)), weights stay on device
# ---------------------------------------------------------------------------

class _Runner:
    def __init__(self):
        import jax
        from jax.sharding import Mesh, NamedSharding, PartitionSpec
        from concourse import bass2jax as b2j
        self.jax = jax
        self.b2j = b2j
        wdt = mybir.dt.bfloat16 if BF16 else F32
        nc = _build(wdt)
        self.nc = nc
        b2j.install_neuronx_cc_hook()
        partition_name = (nc.partition_id_tensor.name
                          if nc.partition_id_tensor else None)
        in_names, out_names, out_avals, zero_outs = [], [], [], []
        for alloc in nc.m.functions[0].allocations:
            if not isinstance(alloc, mybir.MemoryLocationSet):
                continue
            name = alloc.memorylocations[0].name
            if alloc.kind == "ExternalInput":
                if name != partition_name:
                    in_names.append(name)
            elif alloc.kind == "ExternalOutput":
                shape = tuple(alloc.tensor_shape)
                dtype = mybir.dt.np(alloc.dtype)
                out_names.append(name)
                out_avals.append(jax.core.ShapedArray(shape, dtype))
                zero_outs.append(np.zeros((NC_ * shape[0], *shape[1:]), dtype))
        n_params = len(in_names)
        bind_names = list(in_names) + list(out_names)
        if partition_name is not None:
            bind_names.append(partition_name)

        def _body(*args):
            operands = list(args)
            if partition_name is not None:
                operands.append(b2j.partition_id_tensor())
            outs = b2j._bass_exec_p.bind(
                *operands,
                out_avals=tuple(out_avals),
                in_names=tuple(bind_names),
                out_names=tuple(out_names),
                lowering_input_output_aliases=(),
                sim_require_finite=True,
                sim_require_nnan=True,
                nc=nc,
            )
            return tuple(outs)

        devices = jax.devices()[:NC_]
        assert len(devices) == NC_, f"need {NC_} cores, got {len(devices)}"
        mesh = Mesh(np.asarray(devices), ("core",))
        self.sharding = NamedSharding(mesh, PartitionSpec("core"))
        n_outs = len(out_names)
        donate = tuple(range(n_params, n_params + n_outs))
        self.fn = jax.jit(
            b2j.shard_map(_body, mesh=mesh,
                          in_specs=(PartitionSpec("core"),) * (n_params + n_outs),
                          out_specs=(PartitionSpec("core"),) * n_outs,
                          check_rep=False),
            donate_argnums=donate, keep_unused=True)
        self.in_names = in_names
        self.out_names = out_names
        self.zero_outs = zero_outs
        self.dev = {}

    def stage(self, g):
        for name, arr in g.items():
            self.dev[name] = self.jax.device_put(arr, self.sharding)
        for a in self.dev.values():
            a.block_until_ready()

    def step(self, small):
        args = [small[n] if n in small else self.dev[n] for n in self.in_names]
        outs = self.fn(*args, *self.zero_outs)
        return np.asarray(outs[self.out_names.index("logits")])  # [8, VS]

    def bench(self, small, iters):
        """Chained executions: call i+1 consumes call i's donated output
        buffers, so the i executions serialize on device. Returns the wall
        for `iters` marginal executions after one warm call."""
        import time as _time
        jax = self.jax
        sm = {n: jax.device_put(small[n], self.sharding) for n in small}
        args = [sm[n] if n in sm else self.dev[n] for n in self.in_names]
        cur = self.fn(*args, *self.zero_outs)
        jax.block_until_ready(cur)
        t0 = _time.perf_counter_ns()
        for _ in range(iters):
            cur = self.fn(*args, *cur)
        jax.block_until_ready(cur)
        t1 = _time.perf_counter_ns()
        logits = np.asarray(cur[self.out_names.index("logits")])
        return {"wall_ns": t1 - t0, "logits": logits}


# ---------------------------------------------------------------------------
# daemon: device work lives in a respawnable child process, so a wedged /
# crashed PJRT client (NRT_EXEC_UNIT_UNRECOVERABLE has been observed on cold
# first executions in this environment) costs a respawn+retry, not the run.
# ---------------------------------------------------------------------------

import pickle
import shutil
import socket as _socket
import struct
import subprocess
import tempfile
import time
import traceback

_SELF_PATH = os.path.abspath(__file__)


def _send_msg(sock, obj):
    data = pickle.dumps(obj, protocol=5)
    sock.sendall(struct.pack("<Q", len(data)))
    sock.sendall(data)


def _recv_exact(sock, n):
    buf = bytearray()
    while len(buf) < n:
        chunk = sock.recv(min(1 << 20, n - len(buf)))
        if not chunk:
            raise EOFError("daemon pipe closed")
        buf += chunk
    return bytes(buf)


def _recv_msg(sock):
    (n,) = struct.unpack("<Q", _recv_exact(sock, 8))
    return pickle.loads(_recv_exact(sock, n))


def _weights_root():
    base = "/dev/shm" if os.path.isdir("/dev/shm") and os.access(
        "/dev/shm", os.W_OK) else tempfile.gettempdir()
    return os.path.join(base, "gemma3_decode_weights")


def _write_weights_dir(inp, digest):
    """Dump the heavy f32 inputs as raw .npy once per distinct weight set."""
    d = os.path.join(_weights_root(), digest)
    done = os.path.join(d, ".done")
    if os.path.exists(done):
        return d
    root = _weights_root()
    if os.path.isdir(root):  # drop stale weight sets
        for old in os.listdir(root):
            if old != digest:
                shutil.rmtree(os.path.join(root, old), ignore_errors=True)
    os.makedirs(d, exist_ok=True)
    for k in _HEAVY_IN:
        np.save(os.path.join(d, f"{k}.npy"), np.asarray(inp[k]))
    with open(done, "w") as f:
        f.write("ok")
    return d


def _load_weights_dir(d):
    return {k: np.load(os.path.join(d, f"{k}.npy"), mmap_mode="r")
            for k in _HEAVY_IN}


class _Daemon:
    def __init__(self):
        self.proc = None
        self.sock = None
        self.staged_digest = None

    def alive(self):
        return self.proc is not None and self.proc.poll() is None

    def spawn(self):
        self.close()
        a, b = _socket.socketpair(_socket.AF_UNIX, _socket.SOCK_STREAM)
        log = open("/tmp/gemma3_kernel_daemon.log", "ab", buffering=0)
        self.proc = subprocess.Popen(
            [sys.executable, _SELF_PATH, "--serve", str(b.fileno())],
            pass_fds=[b.fileno()], stdin=subprocess.DEVNULL,
            stdout=log, stderr=log)
        b.close()
        log.close()
        self.sock = a
        self.staged_digest = None
        self.rpc({"op": "ping"}, timeout=300)

    def close(self):
        if self.sock is not None:
            try:
                self.sock.close()
            except Exception:
                pass
            self.sock = None
        if self.proc is not None:
            try:
                self.proc.kill()
                self.proc.wait(timeout=10)
            except Exception:
                pass
            self.proc = None
        self.staged_digest = None

    def rpc(self, obj, timeout=1800):
        self.sock.settimeout(timeout)
        _send_msg(self.sock, obj)
        r = _recv_msg(self.sock)
        if not r.get("ok"):
            raise RuntimeError(f"daemon op {obj.get('op')} failed: "
                               f"{r.get('err')}\n{r.get('tb', '')}")
        return r


def _serve(fd):
    sock = _socket.socket(fileno=fd)
    runner = None
    while True:
        try:
            msg = _recv_msg(sock)
        except (EOFError, OSError):
            os._exit(0)
        op = msg.get("op")
        try:
            if op == "ping":
                _send_msg(sock, {"ok": True})
            elif op == "weights":
                if runner is None:
                    runner = _Runner()
                runner.stage(_prep_weights(_load_weights_dir(msg["path"])))
                _send_msg(sock, {"ok": True})
            elif op == "step":
                logits = runner.step(msg["small"])
                _send_msg(sock, {"ok": True, "logits": logits})
            elif op == "bench":
                res = runner.bench(msg["small"], msg["iters"])
                _send_msg(sock, {"ok": True, **res})
            elif op == "exit":
                _send_msg(sock, {"ok": True})
                os._exit(0)
            else:
                _send_msg(sock, {"ok": False, "err": f"bad op {op!r}"})
        except BaseException as e:  # device errors poison the client: exit
            try:
                _send_msg(sock, {"ok": False, "err": repr(e),
                                 "tb": traceback.format_exc()})
            except Exception:
                pass
            os._exit(13)


# ---------------------------------------------------------------------------
# public entry points
# ---------------------------------------------------------------------------

_DAEMON = None
_RUN = None          # in-process fallback runner
_FP = None
_USE_DAEMON = os.environ.get("KDAEMON", "1") == "1"
_RETRY_SLEEPS = (5, 30, 60)


def _digest_of(fp):
    import hashlib
    return hashlib.sha1(repr(fp).encode()).hexdigest()[:12]


def _run_inproc(inp, fp, small):
    global _RUN, _FP
    if _RUN is None:
        _RUN = _Runner()
        _FP = None
    if fp != _FP:
        _RUN.stage(_prep_weights(inp))
        _FP = fp
    return _RUN.step(small)


def _run_daemon(inp, fp, small):
    global _DAEMON
    digest = _digest_of(fp)
    if _DAEMON is None:
        _DAEMON = _Daemon()
    last = None
    for i, pause in enumerate((0,) + _RETRY_SLEEPS):
        if pause:
            time.sleep(pause)
        try:
            if not _DAEMON.alive():
                _DAEMON.spawn()
            if _DAEMON.staged_digest != digest:
                path = _write_weights_dir(inp, digest)
                _DAEMON.rpc({"op": "weights", "path": path})
                _DAEMON.staged_digest = digest
            return _DAEMON.rpc({"op": "step", "small": small})["logits"]
        except Exception as e:
            last = e
            _DAEMON.close()
    raise last


def kernel(**inputs):
    inp = dict(inputs)
    fp = _fingerprint(inp)
    small = _prep_small(inp)
    if _USE_DAEMON:
        try:
            logits = _run_daemon(inp, fp, small)
        except Exception:
            logits = _run_inproc(inp, fp, small)
    else:
        logits = _run_inproc(inp, fp, small)
    logits = logits.reshape(-1)
    idx = int(np.argmax(logits))
    return np.int32(idx), np.float32(logits[idx])


def bench(iters, **inputs):
    """Timing hook for test.py: wall_ns for `iters` chained marginal device
    executions (weights resident), plus the logits they produce."""
    inp = dict(inputs)
    fp = _fingerprint(inp)
    small = _prep_small(inp)
    if _USE_DAEMON and _DAEMON is not None and _DAEMON.alive():
        digest = _digest_of(fp)
        if _DAEMON.staged_digest != digest:
            path = _write_weights_dir(inp, digest)
            _DAEMON.rpc({"op": "weights", "path": path})
            _DAEMON.staged_digest = digest
        r = _DAEMON.rpc({"op": "bench", "small": small, "iters": iters})
        return r["wall_ns"], r["logits"]
    _run_inproc(inp, fp, small)
    r = _RUN.bench(small, iters)
    return r["wall_ns"], r["logits"]


if __name__ == "__main__" and len(sys.argv) >= 3 and sys.argv[1] == "--serve":
    _serve(int(sys.argv[2]))


# revision 16
# speedup vs baseline: 1.3396x; 1.3396x over previous
"""Gemma3 single-token decode on 8 trn2 NeuronCores (tensor-parallel SPMD).

Sharding: attention by head (pairs of cores compute the same head redundantly,
Wo pre-scaled by 0.5 so the 8-way AllReduce sums correctly); FFN 8-way over the
FF dim; lm_head 8-way over vocab with host-side final argmax; KV cache sliced
to the live prefix and replicated; norms computed on every core.

All matvecs use moving-weight matmuls (activation stationary), activations in
fp32, weights optionally bf16 (KBF16=1).

Execution layer: one persistent jit(shard_map(bass_exec)) callable is built per
process; the prepped weight shards are staged onto the 8 cores once (content-
fingerprint cached) and stay resident, so steady-state kernel() calls ship only
the per-token tensors (embedding row, rope row, masks) and fetch the sharded
logits back.
"""
import sys, os
sys.path.insert(0, '/opt/trn_rl_repo')
import numpy as np
import ml_dtypes

import concourse.bass as bass
import concourse.bacc as bacc
import concourse.mybir as mybir
import concourse.tile as tile

L, HID, NCH, D, H, FF, VOCAB = 12, 1152, 9, 256, 4, 6912, 64000
FSH = FF // 8            # 864 ffn rows per core
VS = VOCAB // 8          # 8000 vocab rows per core
SEFF, T = 1024, 8        # live kv prefix (pos=1000 -> 1024), 8 s-tiles
SCALE, EPS = 256.0 ** -0.5, 1e-6
NC_ = 8
F32 = mybir.dt.float32
AF = mybir.ActivationFunctionType
X_AX = mybir.AxisListType.X

BF16 = os.environ.get("KBF16", "1") == "1"
_SIMGELU = os.environ.get("KSIMGELU", "0") == "1"  # CoreSim lacks Gelu_apprx_tanh
WNP = ml_dtypes.bfloat16 if BF16 else np.float32


def _build(wdt):
    nc = bacc.Bacc("TRN2", target_bir_lowering=False, debug=False, num_devices=NC_)
    _eps_t = nc.alloc_sbuf_tensor("const-eps", [128, 1], F32)
    nc.gpsimd.memset(_eps_t.ap(), EPS)
    nc.const_aps.aps[(F32, EPS)] = _eps_t.ap()
    nc.all_engine_barrier()

    def dI(n, sh, dt=F32):
        return nc.dram_tensor(n, sh, dt, kind="ExternalInput").ap()

    h0row = dI("h0row", [1, HID])
    cs = dI("cs", [1, 1024])
    mcol = dI("mcol", [128, 40])
    um_w = dI("um_w", [128, 8], wdt)
    wqkv = dI("wqkv", [L, 3, 128, 2304], wdt)
    wo = dI("wo", [L, 128, 2, HID], wdt)
    ktd = dI("kt", [L, 128, 2, SEFF], wdt)
    vcd = dI("vc", [L, 128, T, D], wdt)
    wgd = dI("wg", [L, 3, 128, 2592], wdt)
    wud = dI("wu", [L, 3, 128, 2592], wdt)
    wdd = dI("wd", [L, 128, 7, HID], wdt)
    lmd = dI("lm", [NCH, 128, VS], wdt)
    logits = nc.dram_tensor("logits", [1, VS], F32, kind="ExternalOutput").ap()

    with tile.TileContext(nc) as tc, \
         tc.tile_pool(name="const", bufs=1) as Pc, \
         tc.tile_pool(name="wqkv", bufs=3) as Pwq, \
         tc.tile_pool(name="wo", bufs=1) as Pwo, \
         tc.tile_pool(name="kt", bufs=1) as Pkt, \
         tc.tile_pool(name="vc", bufs=1) as Pvc, \
         tc.tile_pool(name="wg", bufs=3) as Pwg, \
         tc.tile_pool(name="wu", bufs=3) as Pwu, \
         tc.tile_pool(name="wd", bufs=3) as Pwd, \
         tc.tile_pool(name="lm", bufs=4) as Plm, \
         tc.tile_pool(name="act", bufs=2) as Pa, \
         tc.tile_pool(name="row", bufs=3) as Pr, \
         tc.tile_pool(name="ps", bufs=2, space="PSUM") as Pp, \
         tc.tile_pool(name="dram", bufs=2, space="DRAM") as Pd:

        MM = nc.tensor.matmul
        one_f = Pc.tile([1, 1], F32, tag="onef")
        nc.vector.memset(one_f[:], 1.0)
        one_w = Pc.tile([1, 1], wdt, tag="onew")
        nc.vector.memset(one_w[:], 1.0)
        ones_cf = Pc.tile([128, 1], F32, tag="ocf")
        nc.vector.memset(ones_cf[:], 1.0)
        ones_row = Pc.tile([1, 128], F32, tag="orow")
        nc.vector.memset(ones_row[:], 1.0)
        cs_t = Pc.tile([1, 1024], F32, tag="cs")
        nc.sync.dma_start(out=cs_t[:], in_=cs[:])
        mc = Pc.tile([128, 40], F32, tag="mc")
        nc.sync.dma_start(out=mc[:], in_=mcol[:])
        umw_t = Pc.tile([128, 8], wdt, tag="umw")
        nc.sync.dma_start(out=umw_t[:], in_=um_w[:])
        ADDM, VM, VMU, UM1, UMF = (mc[:, 8 * i:8 * i + 8] for i in range(5))

        def cast_col(src_t, tag):
            if wdt == F32:
                return src_t
            w = Pa.tile([128, NCH], wdt, tag=tag)
            nc.vector.tensor_copy(w[:], src_t[:])
            return w

        def columnize(row_ap, n, one_t, PS, base):
            ps = PS[:, base:base + n]
            for j in range(n):
                MM(ps[:, j:j + 1], row_ap[0:1, j * 128:(j + 1) * 128], one_t[:],
                   start=True, stop=True)
            return ps

        def rms_col(h_t, tag, PS, base):
            sq = Pa.tile([128, NCH], F32, tag="sq")
            nc.vector.tensor_mul(sq[:], h_t[:], h_t[:])
            MM(PS[0:1, base:base + NCH], ones_cf[:], sq[:], start=True, stop=True)
            st = Pa.tile([1, 4], F32, tag="rmsst")
            nc.vector.reduce_sum(st[0:1, 0:1], PS[0:1, base:base + NCH], axis=X_AX)
            nc.scalar.activation(st[0:1, 1:2], st[0:1, 0:1], AF.Sqrt,
                                 bias=EPS, scale=1.0 / HID)
            nc.vector.reciprocal(st[0:1, 2:3], st[0:1, 1:2])
            rb = PS[:, 96 + base:97 + base]
            MM(rb, ones_row[:], st[0:1, 2:3], start=True, stop=True)
            x = Pa.tile([128, NCH], F32, tag=tag)
            nc.vector.tensor_scalar_mul(x[:], h_t[:], rb)
            return x

        def resid_add(h_t, row_t, PS):
            st = Pa.tile([1, 4], F32, tag="rmsst")
            scr = Pr.tile([1, HID], F32, tag="r1152")
            nc.scalar.activation(scr[:], row_t[:], AF.Square,
                                 accum_out=st[0:1, 0:1])
            nc.scalar.activation(st[0:1, 1:2], st[0:1, 0:1], AF.Sqrt,
                                 bias=EPS, scale=1.0 / HID)
            nc.vector.reciprocal(st[0:1, 2:3], st[0:1, 1:2])
            rb = PS[:, 74:75]
            MM(rb, ones_row[:], st[0:1, 2:3], start=True, stop=True)
            pc = columnize(row_t, NCH, one_f, PS, 64)
            tmp = Pa.tile([128, NCH], F32, tag="tmph")
            nc.vector.tensor_scalar_mul(tmp[:], pc[:], rb)
            hn = Pa.tile([128, NCH], F32, tag="h")
            nc.vector.tensor_add(hn[:], h_t[:], tmp[:])
            return hn

        NOAR = os.environ.get("KNOAR", "0") == "1"       # timing probe only
        USE_AG = os.environ.get("KAG", "0") == "1"       # allgather+local reduce

        def all_reduce(row_t):
            bin_ = Pd.tile([1, HID], F32, tag="arin")
            nc.scalar.dma_start(out=bin_[:], in_=row_t[:])
            if NOAR:
                ar = Pr.tile([1, HID], F32, tag="r1152")
                nc.scalar.dma_start(out=ar[:], in_=bin_[:])
                return ar
            if USE_AG:
                bout = Pd.tile([8, HID], F32, tag="arout8")
                nc.gpsimd.collective_compute(
                    "AllGather", mybir.AluOpType.bypass,
                    replica_groups=[list(range(NC_))],
                    ins=[bin_.opt()], outs=[bout.opt()])
                gath = Pa.tile([8, HID], F32, tag="gath")
                nc.sync.dma_start(out=gath[:], in_=bout[:])
                pr = Pp.tile([1, HID], F32, tag="pbig", name=None)
                MM(pr[0:1, 0:512], ones_cf[0:8, :], gath[:, 0:512],
                   start=True, stop=True)
                MM(pr[0:1, 512:1024], ones_cf[0:8, :], gath[:, 512:1024],
                   start=True, stop=True)
                MM(pr[0:1, 1024:1152], ones_cf[0:8, :], gath[:, 1024:1152],
                   start=True, stop=True)
                ar = Pr.tile([1, HID], F32, tag="r1152")
                nc.scalar.activation(ar[:], pr[0:1, :], AF.Copy)
                return ar
            bout = Pd.tile([1, HID], F32, tag="arout")
            nc.gpsimd.collective_compute(
                "AllReduce", mybir.AluOpType.add,
                replica_groups=[list(range(NC_))],
                ins=[bin_.opt()], outs=[bout.opt()])
            ar = Pr.tile([1, HID], F32, tag="r1152")
            nc.scalar.dma_start(out=ar[:], in_=bout[:])
            return ar

        # h0: [1,1152] row -> column layout
        h0r = Pr.tile([1, HID], F32, tag="r1152")
        nc.sync.dma_start(out=h0r[:], in_=h0row[:])
        PS = Pp.tile([128, 512], F32, tag="psmall")
        pc0 = columnize(h0r, NCH, one_f, PS, 64)
        h = Pa.tile([128, NCH], F32, tag="h")
        nc.scalar.activation(h[:], pc0[:], AF.Copy)

        for l in range(L):
            # ---- attention ----
            PS = Pp.tile([128, 512], F32, tag="psmall")
            x = rms_col(h, "x", PS, 0)
            xw = cast_col(x, "xw")
            pqkv = Pp.tile([1, 1152], F32, tag="pbig")
            for g in range(3):
                wt = Pwq.tile([128, 2304], wdt, tag="wqkv")
                nc.sync.dma_start(out=wt[:], in_=wqkv[l, g])
                for ci in range(3):
                    c = g * 3 + ci
                    for n0, ln in ((0, 512), (512, 256)):
                        MM(pqkv[0:1, n0:n0 + ln], xw[:, c:c + 1],
                           wt[:, ci * 768 + n0: ci * 768 + n0 + ln],
                           start=(c == 0), stop=(c == 8))
            # q/k rms over D (rows on partition 0)
            st = Pa.tile([1, 6], F32, tag="qkst")
            scr = Pr.tile([1, 256], F32, tag="r256")
            nc.scalar.activation(scr[:], pqkv[0:1, 0:256], AF.Square,
                                 accum_out=st[0:1, 0:1])
            scr2 = Pr.tile([1, 256], F32, tag="r256")
            nc.scalar.activation(scr2[:], pqkv[0:1, 256:512], AF.Square,
                                 accum_out=st[0:1, 1:2])
            nc.scalar.activation(st[0:1, 2:3], st[0:1, 0:1], AF.Sqrt,
                                 bias=EPS, scale=1.0 / D)
            nc.scalar.activation(st[0:1, 3:4], st[0:1, 1:2], AF.Sqrt,
                                 bias=EPS, scale=1.0 / D)
            nc.vector.reciprocal(st[0:1, 4:5], st[0:1, 2:3])
            nc.vector.reciprocal(st[0:1, 5:6], st[0:1, 3:4])
            cof = 512 if ((l + 1) % 6 == 0) else 0
            cosr = cs_t[0:1, cof:cof + 256]
            sinr = cs_t[0:1, cof + 256:cof + 512]

            def rope(off, rinv, tag):
                t1 = Pr.tile([1, 256], F32, tag="ropet")
                nc.vector.tensor_mul(t1[:], pqkv[0:1, off:off + 256], cosr)
                sw = Pr.tile([1, 256], F32, tag="ropes")
                nc.vector.tensor_copy(sw[0:1, 0:128], pqkv[0:1, off + 128:off + 256])
                nc.vector.tensor_copy(sw[0:1, 128:256], pqkv[0:1, off:off + 128])
                nc.vector.tensor_mul(sw[:], sw[:], sinr)
                nc.vector.tensor_add(t1[:], t1[:], sw[:])
                out = Pr.tile([1, 256], F32, tag=tag)
                nc.vector.tensor_scalar_mul(out[:], t1[:], rinv)
                return out

            qr = rope(0, st[0:1, 4:5], "qr")
            kr = rope(256, st[0:1, 5:6], "kr")
            # columnize q,k -> [128,2] each (wdt)
            pqc = PS[:, 88:92]
            for j in range(2):
                MM(pqc[:, j:j + 1], qr[0:1, j * 128:(j + 1) * 128], one_f[:],
                   start=True, stop=True)
                MM(pqc[:, 2 + j:3 + j], kr[0:1, j * 128:(j + 1) * 128], one_f[:],
                   start=True, stop=True)
            qkc = Pa.tile([128, 4], wdt, tag="qkc")
            nc.scalar.activation(qkc[:], pqc[:], AF.Copy)

            # scores^T [128, T] (s = t*128 + r)
            kt_t = Pkt.tile([128, 2, SEFF], wdt, tag="kt")
            nc.sync.dma_start(out=kt_t[:], in_=ktd[l])
            psc = PS[:, 80:88]
            for t_ in range(T):
                for c in range(2):
                    MM(psc[:, t_:t_ + 1],
                       kt_t[:, c, t_ * 128: t_ * 128 + 128],
                       qkc[:, c:c + 1], start=(c == 0), stop=(c == 1))
            # qk_new = q . k_new
            pqk = PS[0:1, 18:48]
            for c in range(2):
                MM(pqk[0:1, 10:11], qkc[:, c:c + 1], qkc[:, 2 + c:3 + c],
                   start=(c == 0), stop=(c == 1))
            qks = Pa.tile([1, 1], F32, tag="qks")
            nc.scalar.activation(qks[:], pqk[0:1, 10:11], AF.Copy)
            bq = PS[:, 78:79]
            MM(bq, ones_row[:], qks[:], start=True, stop=True)
            # fix scores at s=p, scale, mask, clamp, exp
            sc1 = Pa.tile([128, T], F32, tag="sc1")
            nc.vector.tensor_mul(sc1[:], psc[:], UM1)
            sc2 = Pa.tile([128, T], F32, tag="sc2")
            nc.vector.tensor_scalar_mul(sc2[:], UMF, bq)
            nc.vector.tensor_add(sc1[:], sc1[:], sc2[:])
            nc.vector.tensor_scalar_mul(sc1[:], sc1[:], float(SCALE))
            nc.vector.tensor_add(sc1[:], sc1[:], ADDM)
            nc.vector.tensor_scalar_max(sc1[:], sc1[:], -30.0)
            probs = Pa.tile([128, T], F32, tag="probs")
            nc.scalar.activation(probs[:], sc1[:], AF.Exp)
            # denominator and p_at_update (f32)
            pmf = Pa.tile([128, T], F32, tag="pmf")
            nc.vector.tensor_mul(pmf[:], probs[:], VM)
            puf = Pa.tile([128, T], F32, tag="puf")
            nc.vector.tensor_mul(puf[:], probs[:], UMF)
            MM(pqk[0:1, 0:8], ones_cf[:], pmf[:], start=True, stop=True)
            psums = Pa.tile([1, 8], F32, tag="psums")
            nc.scalar.activation(psums[:], pqk[0:1, 0:8], AF.Copy)
            MM(pqk[0:1, 8:10], ones_cf[:], puf[:, 0:2], start=True, stop=False)
            MM(pqk[0:1, 8:10], ones_cf[:], puf[:, 2:4], start=False, stop=False)
            MM(pqk[0:1, 8:10], ones_cf[:], puf[:, 4:6], start=False, stop=False)
            MM(pqk[0:1, 8:10], ones_cf[:], puf[:, 6:8], start=False, stop=True)
            dn = Pa.tile([1, 4], F32, tag="dn")
            nc.vector.reduce_sum(dn[0:1, 0:1], psums[0:1, 0:8], axis=X_AX)
            nc.vector.reciprocal(dn[0:1, 1:2], dn[0:1, 0:1])
            nc.vector.reduce_sum(dn[0:1, 2:3], pqk[0:1, 8:10], axis=X_AX)
            # o = (probs_masked @ V + pu*v_new) / den
            pmv = Pa.tile([128, T], wdt, tag="pmv")
            nc.vector.tensor_mul(pmv[:], probs[:], VMU)
            vc_t = Pvc.tile([128, T, D], wdt, tag="vc")
            nc.sync.dma_start(out=vc_t[:], in_=vcd[l])
            po = PS[0:1, 128:384]
            for t_ in range(T):
                MM(po[0:1, 0:256], pmv[:, t_:t_ + 1], vc_t[:, t_, :],
                   start=(t_ == 0), stop=(t_ == T - 1))
            vv = Pr.tile([1, 256], F32, tag="vv")
            nc.vector.tensor_scalar_mul(vv[:], pqkv[0:1, 512:768], dn[0:1, 2:3])
            ofin = Pr.tile([1, 256], F32, tag="ofin")
            nc.vector.tensor_add(ofin[:], po[0:1, 0:256], vv[:])
            nc.vector.tensor_scalar_mul(ofin[:], ofin[:], dn[0:1, 1:2])
            # Wo partial (pre-scaled 0.5 on host)
            poc = PS[:, 92:96]
            for j in range(2):
                MM(poc[:, j:j + 1], ofin[0:1, j * 128:(j + 1) * 128], one_f[:],
                   start=True, stop=True)
            ocol = Pa.tile([128, 2], wdt, tag="ocol")
            nc.scalar.activation(ocol[:], poc[:, 92 - 92:94 - 92], AF.Copy)
            wo_t = Pwo.tile([128, 2, HID], wdt, tag="wo")
            nc.sync.dma_start(out=wo_t[:], in_=wo[l])
            prow = Pp.tile([1, HID], F32, tag="pbig")
            for c in range(2):
                for n0, ln in ((0, 512), (512, 512), (1024, 128)):
                    MM(prow[0:1, n0:n0 + ln], ocol[:, c:c + 1],
                       wo_t[:, c, n0: n0 + ln],
                       start=(c == 0), stop=(c == 1))
            arow = Pr.tile([1, HID], F32, tag="r1152")
            nc.scalar.activation(arow[:], prow[0:1, :], AF.Copy)
            ar1 = all_reduce(arow)
            h = resid_add(h, ar1, PS)

            # ---- ffn ----
            x2 = rms_col(h, "x2", PS, 9)
            x2w = cast_col(x2, "x2w")
            pg = Pp.tile([1, FSH], F32, tag="pbig", padded_shape=[1, HID])
            pu_ = Pp.tile([1, FSH], F32, tag="pbig", padded_shape=[1, HID])
            for g in range(3):
                wg_t = Pwg.tile([128, 2592], wdt, tag="wg")
                nc.sync.dma_start(out=wg_t[:], in_=wgd[l, g])
                wu_t = Pwu.tile([128, 2592], wdt, tag="wu")
                nc.sync.dma_start(out=wu_t[:], in_=wud[l, g])
                for ci in range(3):
                    c = g * 3 + ci
                    for n0, ln in ((0, 512), (512, 352)):
                        MM(pg[0:1, n0:n0 + ln], x2w[:, c:c + 1],
                           wg_t[:, ci * FSH + n0: ci * FSH + n0 + ln],
                           start=(c == 0), stop=(c == 8))
                        MM(pu_[0:1, n0:n0 + ln], x2w[:, c:c + 1],
                           wu_t[:, ci * FSH + n0: ci * FSH + n0 + ln],
                           start=(c == 0), stop=(c == 8))
            gact = Pr.tile([1, FSH], F32, tag="gact")
            nc.scalar.activation(gact[:], pg[0:1, :],
                     AF.Tanh if _SIMGELU else AF.Gelu_apprx_tanh)
            prod = Pr.tile([1, 896], wdt, tag="prod")
            nc.vector.memset(prod[0:1, FSH:896], 0.0)
            nc.vector.tensor_mul(prod[0:1, 0:FSH], gact[:], pu_[0:1, :])
            pcd = columnize(prod, 7, one_w, PS, 64)
            pdc = Pa.tile([128, 7], wdt, tag="pdc")
            nc.scalar.activation(pdc[:], pcd[:], AF.Copy)
            pf = Pp.tile([1, HID], F32, tag="pbig")
            for s_ in range(4):
                if s_ < 3:
                    wd_t = Pwd.tile([128, 2, HID], wdt, tag="wd")
                    nc.sync.dma_start(out=wd_t[:],
                                      in_=wdd[l, :, 2 * s_:2 * s_ + 2])
                else:
                    wd_t = Pwd.tile([128, 1, HID], wdt, tag="wd")
                    nc.sync.dma_start(out=wd_t[:], in_=wdd[l, :, 6:7])
                for fi in range(2 if s_ < 3 else 1):
                    fc = 2 * s_ + fi
                    for n0, ln in ((0, 512), (512, 512), (1024, 128)):
                        MM(pf[0:1, n0:n0 + ln], pdc[:, fc:fc + 1],
                           wd_t[:, fi, n0: n0 + ln],
                           start=(fc == 0), stop=(fc == 6))
            frow = Pr.tile([1, HID], F32, tag="r1152")
            nc.scalar.activation(frow[:], pf[0:1, :], AF.Copy)
            ar2 = all_reduce(frow)
            h = resid_add(h, ar2, PS)

        # ---- final norm + lm_head (vocab shard) ----
        PSf = Pp.tile([128, 512], F32, tag="psmall")
        xf = rms_col(h, "xf", PSf, 0)
        xfw = cast_col(xf, "xfw")
        for qt in range(4):
            pva = Pp.tile([1, HID], F32, tag="pbig", name=f"pva{qt}")
            pvb = Pp.tile([1, HID], F32, tag="pbig", name=f"pvb{qt}")
            regs = [pva[0:1, 0:500], pva[0:1, 512:1012],
                    pvb[0:1, 0:500], pvb[0:1, 512:1012]]
            for c in range(NCH):
                lm_t = Plm.tile([128, 2000], wdt, tag="lm")
                nc.sync.dma_start(out=lm_t[:],
                                  in_=lmd[c, :, qt * 2000:(qt + 1) * 2000])
                for vi in range(4):
                    MM(regs[vi], xfw[:, c:c + 1],
                       lm_t[:, vi * 500:(vi + 1) * 500],
                       start=(c == 0), stop=(c == NCH - 1))
            for vi in range(4):
                vg = qt * 4 + vi
                lrow = Pr.tile([1, 500], F32, tag="lrow")
                nc.scalar.activation(lrow[:], regs[vi], AF.Copy)
                nc.scalar.dma_start(out=logits[0:1, vg * 500:(vg + 1) * 500],
                                    in_=lrow[:])

    nc.compile()
    return nc


# ---------------------------------------------------------------------------
# host prep
# ---------------------------------------------------------------------------

def _to_w(x):
    """f32 ndarray -> weight dtype (ml_dtypes astype is SIMD-fast)."""
    if not BF16:
        return np.ascontiguousarray(x, np.float32)
    return np.ascontiguousarray(x, np.float32).astype(WNP)


def _grp3(wT, width):
    """[L,1152,width] -> [L,3,128,3*width] (any dtype)."""
    return np.ascontiguousarray(
        wT.reshape(L, 3, 3, 128, width).transpose(0, 1, 3, 2, 4)
    ).reshape(L, 3, 128, 3 * width)


def _prep_weights(inp):
    """Full weight set -> dict of GLOBAL arrays [8*d0, ...] ready to shard."""
    f32 = np.float32
    Wq = _to_w(inp['Wq'])            # [L,1024,1152]
    Wk = _to_w(inp['Wk'])            # [L,256,1152]
    Wv = _to_w(inp['Wv'])
    Wo = _to_w(inp['Wo'].astype(f32) * f32(0.5))   # [L,1152,1024]
    Wg = _to_w(inp['Wg'])            # [L,6912,1152]
    Wu = _to_w(inp['Wu'])
    Wd = _to_w(inp['Wd'])            # [L,1152,6912]
    lm = _to_w(inp['lm_head'])       # [VOCAB,1152]
    kvc = inp['kv_cache']

    g = {
        "wqkv": np.empty((NC_ * L, 3, 128, 2304), WNP),
        "wo": np.empty((NC_ * L, 128, 2, HID), WNP),
        "kt": np.empty((NC_ * L, 128, 2, SEFF), WNP),
        "vc": np.empty((NC_ * L, 128, T, D), WNP),
        "wg": np.empty((NC_ * L, 3, 128, 2592), WNP),
        "wu": np.empty((NC_ * L, 3, 128, 2592), WNP),
        "wd": np.empty((NC_ * L, 128, 7, HID), WNP),
        "lm": np.empty((NC_ * NCH, 128, VS), WNP),
    }

    # shared KV slices (replicated on every core)
    Kc = kvc[0:L, 0, 0:SEFF, :]                        # [L,S,D] f32
    kt1 = _to_w(np.ascontiguousarray(
        Kc.transpose(0, 2, 1).reshape(L, 2, 128, SEFF).transpose(0, 2, 1, 3)))
    vc1 = _to_w(np.ascontiguousarray(
        kvc[L:2 * L, 0, 0:SEFF, :].reshape(L, T, 128, D).transpose(0, 2, 1, 3)))
    for c in range(NC_):
        g["kt"][c * L:(c + 1) * L] = kt1
        g["vc"][c * L:(c + 1) * L] = vc1

    # attention shards: 4 distinct (head hd = c % 4), reused by core pairs
    for hd in range(4):
        wcat = np.concatenate([Wq[:, hd * D:(hd + 1) * D, :], Wk, Wv], axis=1)
        wqkv1 = _grp3(np.ascontiguousarray(wcat.transpose(0, 2, 1)), 768)
        wo1 = np.ascontiguousarray(
            Wo[:, :, hd * D:(hd + 1) * D].transpose(0, 2, 1)
            .reshape(L, 2, 128, HID).transpose(0, 2, 1, 3))
        for c in (hd, hd + 4):
            g["wqkv"][c * L:(c + 1) * L] = wqkv1
            g["wo"][c * L:(c + 1) * L] = wo1

    # ffn + lm_head shards: distinct per core
    for c in range(NC_):
        sl = slice(c * FSH, (c + 1) * FSH)
        g["wg"][c * L:(c + 1) * L] = _grp3(
            np.ascontiguousarray(Wg[:, sl, :].transpose(0, 2, 1)), FSH)
        g["wu"][c * L:(c + 1) * L] = _grp3(
            np.ascontiguousarray(Wu[:, sl, :].transpose(0, 2, 1)), FSH)
        wdT = np.zeros((L, 896, HID), WNP)
        wdT[:, :FSH, :] = Wd[:, :, sl].transpose(0, 2, 1)
        g["wd"][c * L:(c + 1) * L] = np.ascontiguousarray(
            wdT.reshape(L, 7, 128, HID).transpose(0, 2, 1, 3))
        g["lm"][c * NCH:(c + 1) * NCH] = np.ascontiguousarray(
            lm[c * VS:(c + 1) * VS, :].T).reshape(NCH, 128, VS)
    return g


def _prep_small(inp):
    """Per-token tensors -> dict of GLOBAL arrays (replicated across cores)."""
    f32 = np.float32
    p = int(np.asarray(inp['position_ids'])[0])
    tok = int(np.asarray(inp['input_ids'])[0])
    assert p + 1 <= SEFF, f"position {p} exceeds compiled kv window {SEFF}"

    h0 = (np.asarray(inp['embed'][tok]).astype(f32)
          * f32(HID ** 0.5)).reshape(1, HID)

    def sinsig(s):
        s = np.asarray(s)
        return np.concatenate([-s[0:128], s[128:256]])

    cs = np.concatenate([
        np.asarray(inp['cos_sliding'][p]), sinsig(inp['sin_sliding'][p]),
        np.asarray(inp['cos_full'][p]), sinsig(inp['sin_full'][p])
    ]).astype(f32).reshape(1, 1024)

    cm = np.asarray(inp['causal_mask'][:SEFF]).astype(f32)
    um = np.asarray(inp['update_mask'][:SEFF, 0]).astype(f32)
    col = lambda a: np.ascontiguousarray(a.reshape(T, 128).T)
    addm, umc = col(cm), col(um)
    vm = (addm > -1.0).astype(f32)
    mcol = np.concatenate([addm, vm, vm * (1 - umc), 1 - umc, umc],
                          axis=1).astype(f32)
    um_w = umc.astype(WNP)

    def rep(a):
        return np.ascontiguousarray(
            np.broadcast_to(a, (NC_, *a.shape))).reshape(NC_ * a.shape[0],
                                                         *a.shape[1:])

    return {"h0row": rep(h0), "cs": rep(cs), "mcol": rep(mcol),
            "um_w": rep(um_w)}


_HEAVY_IN = ("Wq", "Wk", "Wv", "Wo", "Wg", "Wu", "Wd", "lm_head", "kv_cache")


def _fingerprint(inp):
    """Content fingerprint of the weight tensors (strided sample + edges)."""
    parts = []
    for k in _HEAVY_IN:
        a = np.asarray(inp[k])
        v = a.reshape(-1).view(np.uint64) if a.nbytes % 8 == 0 else \
            a.reshape(-1).view(np.uint8)
        s = int(v[::61].sum(dtype=np.uint64))
        e = int(v[:512].sum(dtype=np.uint64)) ^ int(v[-512:].sum(dtype=np.uint64))
        parts.append((k, a.shape, a.dtype.str, s, e))
    return tuple(parts)


# ---------------------------------------------------------------------------
# persistent executor: one jit(shard_map(bass_exec)), weights stay on device
# ---------------------------------------------------------------------------

class _Runner:
    def __init__(self):
        import jax
        from jax.sharding import Mesh, NamedSharding, PartitionSpec
        from concourse import bass2jax as b2j
        self.jax = jax
        self.b2j = b2j
        wdt = mybir.dt.bfloat16 if BF16 else F32
        nc = _build(wdt)
        self.nc = nc
        b2j.install_neuronx_cc_hook()
        partition_name = (nc.partition_id_tensor.name
                          if nc.partition_id_tensor else None)
        in_names, out_names, out_avals, zero_outs = [], [], [], []
        for alloc in nc.m.functions[0].allocations:
            if not isinstance(alloc, mybir.MemoryLocationSet):
                continue
            name = alloc.memorylocations[0].name
            if alloc.kind == "ExternalInput":
                if name != partition_name:
                    in_names.append(name)
            elif alloc.kind == "ExternalOutput":
                shape = tuple(alloc.tensor_shape)
                dtype = mybir.dt.np(alloc.dtype)
                out_names.append(name)
                out_avals.append(jax.core.ShapedArray(shape, dtype))
                zero_outs.append(np.zeros((NC_ * shape[0], *shape[1:]), dtype))
        n_params = len(in_names)
        bind_names = list(in_names) + list(out_names)
        if partition_name is not None:
            bind_names.append(partition_name)

        def _body(*args):
            operands = list(args)
            if partition_name is not None:
                operands.append(b2j.partition_id_tensor())
            outs = b2j._bass_exec_p.bind(
                *operands,
                out_avals=tuple(out_avals),
                in_names=tuple(bind_names),
                out_names=tuple(out_names),
                lowering_input_output_aliases=(),
                sim_require_finite=True,
                sim_require_nnan=True,
                nc=nc,
            )
            return tuple(outs)

        devices = jax.devices()[:NC_]
        assert len(devices) == NC_, f"need {NC_} cores, got {len(devices)}"
        mesh = Mesh(np.asarray(devices), ("core",))
        self.sharding = NamedSharding(mesh, PartitionSpec("core"))
        n_outs = len(out_names)
        donate = tuple(range(n_params, n_params + n_outs))
        self.fn = jax.jit(
            b2j.shard_map(_body, mesh=mesh,
                          in_specs=(PartitionSpec("core"),) * (n_params + n_outs),
                          out_specs=(PartitionSpec("core"),) * n_outs,
                          check_rep=False),
            donate_argnums=donate, keep_unused=True)
        self.in_names = in_names
        self.out_names = out_names
        self.zero_outs = zero_outs
        self.dev = {}

    def stage(self, g):
        for name, arr in g.items():
            self.dev[name] = self.jax.device_put(arr, self.sharding)
        for a in self.dev.values():
            a.block_until_ready()

    def step(self, small):
        args = [small[n] if n in small else self.dev[n] for n in self.in_names]
        outs = self.fn(*args, *self.zero_outs)
        return np.asarray(outs[self.out_names.index("logits")])  # [8, VS]

    def bench(self, small, iters):
        """Chained executions: call i+1 consumes call i's donated output
        buffers, so the i executions serialize on device. Returns the wall
        for `iters` marginal executions after one warm call."""
        import time as _time
        jax = self.jax
        sm = {n: jax.device_put(small[n], self.sharding) for n in small}
        args = [sm[n] if n in sm else self.dev[n] for n in self.in_names]
        cur = self.fn(*args, *self.zero_outs)
        jax.block_until_ready(cur)
        t0 = _time.perf_counter_ns()
        for _ in range(iters):
            cur = self.fn(*args, *cur)
        jax.block_until_ready(cur)
        t1 = _time.perf_counter_ns()
        logits = np.asarray(cur[self.out_names.index("logits")])
        return {"wall_ns": t1 - t0, "logits": logits}


# ---------------------------------------------------------------------------
# daemon: device work lives in a respawnable child process, so a wedged /
# crashed PJRT client (NRT_EXEC_UNIT_UNRECOVERABLE has been observed on cold
# first executions in this environment) costs a respawn+retry, not the run.
# ---------------------------------------------------------------------------

import pickle
import shutil
import socket as _socket
import struct
import subprocess
import tempfile
import time
import traceback

_SELF_PATH = os.path.abspath(__file__)


def _send_msg(sock, obj):
    data = pickle.dumps(obj, protocol=5)
    sock.sendall(struct.pack("<Q", len(data)))
    sock.sendall(data)


def _recv_exact(sock, n):
    buf = bytearray()
    while len(buf) < n:
        chunk = sock.recv(min(1 << 20, n - len(buf)))
        if not chunk:
            raise EOFError("daemon pipe closed")
        buf += chunk
    return bytes(buf)


def _recv_msg(sock):
    (n,) = struct.unpack("<Q", _recv_exact(sock, 8))
    return pickle.loads(_recv_exact(sock, n))


def _weights_root():
    base = "/dev/shm" if os.path.isdir("/dev/shm") and os.access(
        "/dev/shm", os.W_OK) else tempfile.gettempdir()
    return os.path.join(base, "gemma3_decode_weights")


def _write_weights_dir(inp, digest):
    """Dump the heavy f32 inputs as raw .npy once per distinct weight set."""
    d = os.path.join(_weights_root(), digest)
    done = os.path.join(d, ".done")
    if os.path.exists(done):
        return d
    root = _weights_root()
    if os.path.isdir(root):  # drop stale weight sets
        for old in os.listdir(root):
            if old != digest:
                shutil.rmtree(os.path.join(root, old), ignore_errors=True)
    os.makedirs(d, exist_ok=True)
    for k in _HEAVY_IN:
        np.save(os.path.join(d, f"{k}.npy"), np.asarray(inp[k]))
    with open(done, "w") as f:
        f.write("ok")
    return d


def _load_weights_dir(d):
    return {k: np.load(os.path.join(d, f"{k}.npy"), mmap_mode="r")
            for k in _HEAVY_IN}


class _Daemon:
    def __init__(self):
        self.proc = None
        self.sock = None
        self.staged_digest = None

    def alive(self):
        return self.proc is not None and self.proc.poll() is None

    def spawn(self):
        self.close()
        a, b = _socket.socketpair(_socket.AF_UNIX, _socket.SOCK_STREAM)
        log = open("/tmp/gemma3_kernel_daemon.log", "ab", buffering=0)
        self.proc = subprocess.Popen(
            [sys.executable, _SELF_PATH, "--serve", str(b.fileno())],
            pass_fds=[b.fileno()], stdin=subprocess.DEVNULL,
            stdout=log, stderr=log)
        b.close()
        log.close()
        self.sock = a
        self.staged_digest = None
        self.rpc({"op": "ping"}, timeout=300)

    def close(self):
        if self.sock is not None:
            try:
                self.sock.close()
            except Exception:
                pass
            self.sock = None
        if self.proc is not None:
            try:
                self.proc.kill()
                self.proc.wait(timeout=10)
            except Exception:
                pass
            self.proc = None
        self.staged_digest = None

    def rpc(self, obj, timeout=1800):
        self.sock.settimeout(timeout)
        _send_msg(self.sock, obj)
        r = _recv_msg(self.sock)
        if not r.get("ok"):
            raise RuntimeError(f"daemon op {obj.get('op')} failed: "
                               f"{r.get('err')}\n{r.get('tb', '')}")
        return r


def _serve(fd):
    sock = _socket.socket(fileno=fd)
    runner = None
    while True:
        try:
            msg = _recv_msg(sock)
        except (EOFError, OSError):
            os._exit(0)
        op = msg.get("op")
        try:
            if op == "ping":
                _send_msg(sock, {"ok": True})
            elif op == "weights":
                if runner is None:
                    runner = _Runner()
                runner.stage(_prep_weights(_load_weights_dir(msg["path"])))
                _send_msg(sock, {"ok": True})
            elif op == "step":
                logits = runner.step(msg["small"])
                _send_msg(sock, {"ok": True, "logits": logits})
            elif op == "bench":
                res = runner.bench(msg["small"], msg["iters"])
                _send_msg(sock, {"ok": True, **res})
            elif op == "exit":
                _send_msg(sock, {"ok": True})
                os._exit(0)
            else:
                _send_msg(sock, {"ok": False, "err": f"bad op {op!r}"})
        except BaseException as e:  # device errors poison the client: exit
            try:
                _send_msg(sock, {"ok": False, "err": repr(e),
                                 "tb": traceback.format_exc()})
            except Exception:
                pass
            os._exit(13)


# ---------------------------------------------------------------------------
# public entry points
# ---------------------------------------------------------------------------

_DAEMON = None
_RUN = None          # in-process fallback runner
_FP = None
_USE_DAEMON = os.environ.get("KDAEMON", "1") == "1"
_RETRY_SLEEPS = (5, 30, 60)


def _digest_of(fp):
    import hashlib
    return hashlib.sha1(repr(fp).encode()).hexdigest()[:12]


def _run_inproc(inp, fp, small):
    global _RUN, _FP
    if _RUN is None:
        _RUN = _Runner()
        _FP = None
    if fp != _FP:
        _RUN.stage(_prep_weights(inp))
        _FP = fp
    return _RUN.step(small)


def _run_daemon(inp, fp, small):
    global _DAEMON
    digest = _digest_of(fp)
    if _DAEMON is None:
        _DAEMON = _Daemon()
    last = None
    for i, pause in enumerate((0,) + _RETRY_SLEEPS):
        if pause:
            time.sleep(pause)
        try:
            if not _DAEMON.alive():
                _DAEMON.spawn()
            if _DAEMON.staged_digest != digest:
                path = _write_weights_dir(inp, digest)
                _DAEMON.rpc({"op": "weights", "path": path})
                _DAEMON.staged_digest = digest
            return _DAEMON.rpc({"op": "step", "small": small})["logits"]
        except Exception as e:
            last = e
            _DAEMON.close()
    raise last


def kernel(**inputs):
    inp = dict(inputs)
    fp = _fingerprint(inp)
    small = _prep_small(inp)
    if _USE_DAEMON:
        try:
            logits = _run_daemon(inp, fp, small)
        except Exception:
            logits = _run_inproc(inp, fp, small)
    else:
        logits = _run_inproc(inp, fp, small)
    logits = logits.reshape(-1)
    idx = int(np.argmax(logits))
    return np.int32(idx), np.float32(logits[idx])


def bench(iters, **inputs):
    """Timing hook for test.py: wall_ns for `iters` chained marginal device
    executions (weights resident), plus the logits they produce."""
    inp = dict(inputs)
    fp = _fingerprint(inp)
    small = _prep_small(inp)
    if _USE_DAEMON and _DAEMON is not None and _DAEMON.alive():
        digest = _digest_of(fp)
        if _DAEMON.staged_digest != digest:
            path = _write_weights_dir(inp, digest)
            _DAEMON.rpc({"op": "weights", "path": path})
            _DAEMON.staged_digest = digest
        r = _DAEMON.rpc({"op": "bench", "small": small, "iters": iters})
        return r["wall_ns"], r["logits"]
    _run_inproc(inp, fp, small)
    r = _RUN.bench(small, iters)
    return r["wall_ns"], r["logits"]


if __name__ == "__main__" and len(sys.argv) >= 3 and sys.argv[1] == "--serve":
    _serve(int(sys.argv[2]))
